# revision 16
# baseline (speedup 1.0000x reference)
"""Trainium2 Bass kernel for GAT+GCN+MLP message passing (8 NeuronCores, SPMD).

Strategy (dst-node sharding):
  - Host: add self-loops, greedily pack the 10000 nodes into 8 cores x 10
    tiles x 128 slots balancing in-edge counts; build per-edge one-hot dst
    masks (plus GCN-norm-weighted variants) and the edge-expanded x operands
    for both edge endpoints (graph partitioning metadata / layout prep).
  - GAT: e = leaky(a_s[src]+a_d[dst]) computed per edge-chunk entirely with
    matmuls (a_s via x_srcT @ w_as, a_d via x_dstT @ w_ad); messages
    aggregated per head with one-hot mask matmuls into PSUM; normalized by
    the PSUM-accumulated softmax denominators; per-head W applied after.
  - AllGather the bf16 hidden h across the 8 cores; GCN gathers h rows with
    per-chunk indirect DMAs and aggregates with norm-weighted mask matmuls;
    the dense MLP runs feature-major (weight chunks stationary as lhsT)
    with biases folded into Lrelu activations.
"""

import os
import sys
import heapq
import dataclasses

for _p in ("/opt/trn_rl_repo", "/root/.axon_site/_ro/trn_rl_repo"):
    if os.path.isdir(_p) and _p not in sys.path:
        sys.path.insert(0, _p)

import numpy as np
import ml_dtypes

import concourse.bass as bass
import concourse.tile as tile
from concourse import bacc, mybir
from concourse.bass import IndirectOffsetOnAxis
from concourse.bass_utils import run_bass_kernel_spmd

BF16 = ml_dtypes.bfloat16

N = 10000
F_IN = 66
HEADS = 10
F_HEAD = 66
F_GAT = HEADS * F_HEAD          # 660
GCN_OUT = 1320
NCORE = 8
TILES_PER_CORE = 10
NTILE = NCORE * TILES_PER_CORE  # 80
NSLOT = NTILE * 128             # 10240
SLOTS_PER_CORE = TILES_PER_CORE * 128  # 1280

F32 = mybir.dt.float32
BF = mybir.dt.bfloat16
I32 = mybir.dt.int32

_CACHE = {}


# ---------------------------------------------------------------- host prep

def _prep(x, edge_index):
    src = np.concatenate([edge_index[0], np.arange(N, dtype=np.int64)])
    dst = np.concatenate([edge_index[1], np.arange(N, dtype=np.int64)])
    deg = np.bincount(dst, minlength=N).astype(np.int64)

    # pack nodes into 80 tiles of <=128 slots, balancing in-edge counts
    order = np.argsort(-deg, kind="stable")
    tile_cnt = np.zeros(NTILE, np.int64)
    slot = np.empty(N, np.int64)
    hp = [(0, t) for t in range(NTILE)]
    heapq.heapify(hp)
    for n_ in order:
        while True:
            e, t = heapq.heappop(hp)
            if tile_cnt[t] < 128:
                break
        slot[n_] = t * 128 + tile_cnt[t]
        tile_cnt[t] += 1
        heapq.heappush(hp, (e + int(deg[n_]), t))
    sslot = slot[src]
    dslot = slot[dst]
    dtile = dslot >> 7
    dlocal = dslot & 127

    tile_edges = np.bincount(dtile, minlength=NTILE)
    nc_t = int(np.max((tile_edges + 127) // 128))
    e_tile = nc_t * 128
    nidx = TILES_PER_CORE * e_tile
    nchunks = TILES_PER_CORE * nc_t

    esrc = np.zeros((NCORE, TILES_PER_CORE, e_tile), np.int64)
    edstl = np.full((NCORE, TILES_PER_CORE, e_tile), -1, np.int64)
    edst = np.zeros((NCORE, TILES_PER_CORE, e_tile), np.int64)
    ord_t = np.argsort(dtile, kind="stable")
    bounds = np.searchsorted(dtile[ord_t], np.arange(NTILE + 1))
    for t in range(NTILE):
        idx = ord_t[bounds[t]:bounds[t + 1]]
        k = len(idx)
        c, tt = divmod(t, TILES_PER_CORE)
        esrc[c, tt, :k] = sslot[idx]
        edstl[c, tt, :k] = dlocal[idx]
        edst[c, tt, :k] = dslot[idx]

    # one-hot masks [core][128 edge-part, nchunks*128]
    onehot = (edstl[..., None] == np.arange(128))      # [C,T,e_tile,128] bool
    oh = onehot.reshape(NCORE, TILES_PER_CORE, nc_t, 128, 128)
    masks = np.ascontiguousarray(
        oh.transpose(0, 3, 1, 2, 4)).reshape(
        NCORE, 128, nchunks * 128).astype(BF16)

    # norm-weighted masks for GCN: w = dinv[src]*dinv[dst] folded in
    dinv_slot = np.ones(NSLOT, np.float32)
    dinv_slot[slot] = 1.0 / np.sqrt(np.maximum(deg, 1).astype(np.float32))
    wvals = (dinv_slot[esrc] * dinv_slot[edst]).astype(np.float32)
    wm = oh.astype(np.float32) * wvals.reshape(
        NCORE, TILES_PER_CORE, nc_t, 128)[..., None]
    wmasks = np.ascontiguousarray(
        wm.transpose(0, 3, 1, 2, 4)).reshape(
        NCORE, 128, nchunks * 128).astype(BF16)

    # edge-expanded x operands (src- and dst-side)
    x_pad = np.zeros((NSLOT, F_IN), np.float32)
    x_pad[slot] = x
    xg = np.empty((NCORE, 128, nchunks * F_IN), BF16)
    xgT = np.empty((NCORE, F_IN, nchunks * 128), BF16)
    xdT = np.empty((NCORE, F_IN, nchunks * 128), BF16)
    for c in range(NCORE):
        arr = x_pad[esrc[c].reshape(-1)]               # [nidx, 66] f32
        a3 = arr.reshape(nchunks, 128, F_IN)
        xg[c] = np.ascontiguousarray(
            a3.transpose(1, 0, 2)).reshape(128, nchunks * F_IN).astype(BF16)
        xgT[c] = np.ascontiguousarray(arr.T).astype(BF16)
        arrd = x_pad[edst[c].reshape(-1)]              # [nidx, 66] f32
        xdT[c] = np.ascontiguousarray(arrd.T).astype(BF16)

    sidx = np.empty((NCORE, 128, nchunks), np.int32)
    for c in range(NCORE):
        a = esrc[c].reshape(TILES_PER_CORE, nc_t, 128).transpose(2, 0, 1)
        sidx[c] = a.reshape(128, nchunks)

    return dict(slot=slot, nc_t=nc_t, nidx=nidx, masks=masks,
                wmasks=wmasks, xg=xg, xgT=xgT, xdT=xdT, sidx=sidx)


def _prep_weights(W_gat, att_src, att_dst, b_gat, W_gcn, b_gcn,
                  W_g1, b_g1, W_g2, b_g2, W_fc1, b_fc1, W_fc2, b_fc2,
                  W_out, b_out):
    Wg = np.asarray(W_gat, np.float32).reshape(F_IN, HEADS, F_HEAD)
    w_as = np.einsum("fhg,hg->fh", Wg, np.asarray(att_src, np.float32))
    w_ad = np.einsum("fhg,hg->fh", Wg, np.asarray(att_dst, np.float32))
    w_as_bf = np.ascontiguousarray(w_as.astype(BF16))             # [66,10]
    w_ad_bf = np.ascontiguousarray(w_ad.astype(BF16))             # [66,10]

    def chunk_pack(W, kchunks, ncols):
        W = np.asarray(W, np.float32)
        K, M = W.shape
        out = np.zeros((128, kchunks * ncols), BF16)
        for kt in range(kchunks):
            r0 = kt * 128
            r1 = min(K, r0 + 128)
            if r0 >= K:
                break
            out[:r1 - r0, kt * ncols:kt * ncols + M] = W[r0:r1].astype(BF16)
        return out

    W_gcn_p = chunk_pack(W_gcn, 6, GCN_OUT)
    W_g1_p = chunk_pack(W_g1, 11, 1000)
    W_g2_p = chunk_pack(W_g2, 8, 64)

    def col_pack(b, nch):
        out = np.zeros((128, nch), np.float32)
        b = np.asarray(b, np.float32).reshape(-1)
        for mc in range(nch):
            r0 = mc * 128
            r1 = min(b.shape[0], r0 + 128)
            if r0 >= b.shape[0]:
                break
            out[:r1 - r0, mc] = b[r0:r1]
        return out

    b_gcn_col = col_pack(b_gcn, 11)
    b_g1_col = col_pack(b_g1, 8)
    W_fc1_p = np.asarray(W_fc1, BF16)
    W_fc2_p = np.asarray(W_fc2, BF16)
    W_out_p = np.asarray(W_out, BF16)
    b_tail = np.zeros((128, 4), np.float32)
    b_tail[:64, 0] = np.asarray(b_g2, np.float32)
    b_tail[:32, 1] = np.asarray(b_fc1, np.float32)
    b_tail[:16, 2] = np.asarray(b_fc2, np.float32)
    b_tail[0, 3] = float(np.asarray(b_out).reshape(-1)[0])

    ident = np.eye(128, dtype=BF16)
    ones_row = np.ones((1, 512), BF16)
    b_gat_row = np.zeros((1, F_GAT), BF16)
    b_gat_row[0, :] = np.asarray(b_gat, BF16)
    W_heads = np.asarray(W_gat, BF16)

    return dict(w_as_bf=w_as_bf, w_ad_bf=w_ad_bf, W_gcn_p=W_gcn_p,
                W_g1_p=W_g1_p, W_g2_p=W_g2_p, b_gcn_col=b_gcn_col,
                b_g1_col=b_g1_col, W_fc1_p=W_fc1_p, W_fc2_p=W_fc2_p,
                W_out_p=W_out_p, b_tail=b_tail, ident=ident,
                ones_row=ones_row, b_gat_row=b_gat_row, W_heads=W_heads)


def make_in_maps(prep, wts):
    shared = {k: wts[k] for k in
              ["w_as_bf", "w_ad_bf", "W_heads", "b_gat_row", "ones_row",
               "ident", "W_gcn_p", "W_g1_p", "W_g2_p", "b_gcn_col",
               "b_g1_col", "W_fc1_p", "W_fc2_p", "W_out_p", "b_tail"]}
    in_maps = []
    for c in range(NCORE):
        m = dict(shared)
        for k in ["xg", "xgT", "xdT", "masks", "wmasks", "sidx"]:
            m[k] = prep[k][c]
        in_maps.append(m)
    return in_maps


# ---------------------------------------------------------------- device kernel

def _bc(ap, pattern):
    """Replace the free dims of a (sliced) AP with explicit [step,count] dims."""
    return dataclasses.replace(
        ap, ap=[list(ap.ap[0])] + [list(p) for p in pattern])


def _build(nc_t, repeat=1, rep_phases=("B", "AG", "C")):
    nidx = TILES_PER_CORE * nc_t * 128
    nchunks = TILES_PER_CORE * nc_t
    rep_phases = set(rep_phases)

    nc = bacc.Bacc("TRN2", target_bir_lowering=False, debug=False,
                   num_devices=NCORE)

    def inp(name, shape, dt):
        return nc.dram_tensor(name, list(shape), dt, kind="ExternalInput")

    xg_d = inp("xg", [128, nchunks * F_IN], BF)
    xgT_d = inp("xgT", [F_IN, nchunks * 128], BF)
    xdT_d = inp("xdT", [F_IN, nchunks * 128], BF)
    masks_d = inp("masks", [128, nchunks * 128], BF)
    wmasks_d = inp("wmasks", [128, nchunks * 128], BF)
    sidx_d = inp("sidx", [128, nchunks], I32)
    w_as_bf_d = inp("w_as_bf", [F_IN, HEADS], BF)
    w_ad_bf_d = inp("w_ad_bf", [F_IN, HEADS], BF)
    W_heads_d = inp("W_heads", [F_IN, F_GAT], BF)
    b_gat_row_d = inp("b_gat_row", [1, F_GAT], BF)
    ones_row_d = inp("ones_row", [1, 512], BF)
    ident_d = inp("ident", [128, 128], BF)
    W_gcn_d = inp("W_gcn_p", [128, 6 * GCN_OUT], BF)
    W_g1_d = inp("W_g1_p", [128, 11 * 1000], BF)
    W_g2_d = inp("W_g2_p", [128, 8 * 64], BF)
    b_gcn_col_d = inp("b_gcn_col", [128, 11], F32)
    b_g1_col_d = inp("b_g1_col", [128, 8], F32)
    W_fc1_d = inp("W_fc1_p", [64, 32], BF)
    W_fc2_d = inp("W_fc2_p", [32, 16], BF)
    W_out_d = inp("W_out_p", [16, 1], BF)
    b_tail_d = inp("b_tail", [128, 4], F32)

    y_d = nc.dram_tensor("y", [1, SLOTS_PER_CORE], F32, kind="ExternalOutput")

    core_ids = list(range(NCORE))
    AF = mybir.ActivationFunctionType
    OP = mybir.AluOpType

    with tile.TileContext(nc) as tc:
        with tc.tile_pool(name="persist", bufs=1) as pp, \
             tc.tile_pool(name="dram", bufs=1, space="DRAM") as dram:

            h_my = dram.tile([SLOTS_PER_CORE, F_GAT], BF)
            h_full = dram.tile([NSLOT, F_GAT], BF)

            sidx_sb = pp.tile([128, nchunks], I32)
            ident_sb = pp.tile([128, 128], BF)
            ones_sb = pp.tile([1, 512], BF)
            nc.sync.dma_start(sidx_sb[:], sidx_d[:])
            nc.sync.dma_start(ident_sb[:], ident_d[:])
            nc.sync.dma_start(ones_sb[:], ones_row_d[:])

            for _rep in range(repeat):
                # ---------------- phase B: GAT ----------------
                if _rep == 0 or "B" in rep_phases:
                  with tc.tile_pool(name="phaseB", bufs=1) as pb, \
                     tc.tile_pool(name="gat_work", bufs=4) as gw, \
                     tc.tile_pool(name="psumE", bufs=2,
                                  space=bass.MemorySpace.PSUM) as pse, \
                     tc.tile_pool(name="psumG", bufs=1,
                                  space=bass.MemorySpace.PSUM) as psg, \
                     tc.tile_pool(name="psumH", bufs=1,
                                  space=bass.MemorySpace.PSUM) as psh, \
                     tc.tile_pool(name="psumT", bufs=2,
                                  space=bass.MemorySpace.PSUM) as pst:

                    was_sb = pb.tile([F_IN, HEADS], BF)
                    nc.sync.dma_start(was_sb[:], w_as_bf_d[:])
                    wad_sb = pb.tile([F_IN, HEADS], BF)
                    nc.sync.dma_start(wad_sb[:], w_ad_bf_d[:])
                    xgT_sb = pb.tile([F_IN, nchunks * 128], BF)
                    xdT_sb = pb.tile([F_IN, nchunks * 128], BF)
                    xg_sb = pb.tile([128, nchunks * F_IN], BF)
                    masks_sb = pb.tile([128, nchunks * 128], BF)
                    half = (nchunks // 2) * 128
                    halfg = (nchunks // 2) * F_IN
                    nc.sync.dma_start(xgT_sb[:, 0:half], xgT_d[:, 0:half])
                    nc.sync.dma_start(xdT_sb[:, 0:half], xdT_d[:, 0:half])
                    nc.sync.dma_start(xg_sb[:, 0:halfg], xg_d[:, 0:halfg])
                    nc.sync.dma_start(masks_sb[:, 0:half], masks_d[:, 0:half])
                    nc.sync.dma_start(xgT_sb[:, half:], xgT_d[:, half:])
                    nc.sync.dma_start(xdT_sb[:, half:], xdT_d[:, half:])
                    nc.sync.dma_start(xg_sb[:, halfg:], xg_d[:, halfg:])
                    nc.sync.dma_start(masks_sb[:, half:], masks_d[:, half:])
                    W_heads_sb = pb.tile([F_IN, F_GAT], BF)
                    nc.sync.dma_start(W_heads_sb[:], W_heads_d[:])
                    b_gat_sb = pb.tile([1, F_GAT], BF)
                    nc.sync.dma_start(b_gat_sb[:], b_gat_row_d[:])

                    exb = pb.tile([128, nchunks * HEADS], BF)

                    for t in range(TILES_PER_CORE):
                        # e-values for this tile's chunks
                        for k in range(nc_t):
                            c = t * nc_t + k
                            ps_e = pse.tile([128, HEADS], F32, tag="pse")
                            nc.tensor.matmul(ps_e[:],
                                             xgT_sb[:, 128 * c:128 * (c + 1)],
                                             was_sb[:], start=True, stop=False)
                            nc.tensor.matmul(ps_e[:],
                                             xdT_sb[:, 128 * c:128 * (c + 1)],
                                             wad_sb[:], start=False, stop=True)
                            eu = gw.tile([128, HEADS], F32, tag="eu")
                            nc.vector.tensor_scalar(eu[:], ps_e[:], 0.2, None,
                                                    OP.mult)
                            ev = gw.tile([128, HEADS], F32, tag="ev")
                            nc.vector.tensor_tensor(ev[:], ps_e[:], eu[:],
                                                    OP.max)
                            nc.scalar.activation(
                                exb[:, HEADS * c:HEADS * (c + 1)], ev[:],
                                AF.Exp)

                        # psum_g: A-block at cols [0:660), s at [660:670)
                        psum_g = psg.tile([128, 670], F32, tag="psg")
                        for k in range(nc_t):
                            c = t * nc_t + k
                            rhs = gw.tile([128, 670], BF, tag="rhs")
                            xg_b = _bc(xg_sb[:, F_IN * c:F_IN * c + 1],
                                       [[0, HEADS], [1, F_HEAD]])
                            ex_b = _bc(exb[:, HEADS * c:HEADS * c + 1],
                                       [[1, HEADS], [0, F_HEAD]])
                            nc.vector.tensor_tensor(
                                rhs[:, 0:F_GAT].rearrange("p (h f) -> p h f",
                                                          h=HEADS),
                                xg_b, ex_b, OP.mult)
                            nc.vector.tensor_copy(
                                rhs[:, 660:670],
                                exb[:, HEADS * c:HEADS * (c + 1)])
                            mask = masks_sb[:, 128 * c:128 * (c + 1)]
                            st, sp = (k == 0), (k == nc_t - 1)
                            nc.tensor.matmul(psum_g[:, 0:512], mask,
                                             rhs[:, 0:512], start=st, stop=sp)
                            nc.tensor.matmul(psum_g[:, 512:670], mask,
                                             rhs[:, 512:670], start=st, stop=sp)
                        s_sb = gw.tile([128, HEADS], F32, tag="s")
                        nc.vector.tensor_scalar(s_sb[:], psum_g[:, 660:670],
                                                1e-6, None, OP.max)
                        rs = gw.tile([128, HEADS], F32, tag="rs")
                        nc.vector.reciprocal(rs[:], s_sb[:])
                        A_norm = gw.tile([128, 704], BF, tag="anorm")
                        nc.vector.memset(A_norm[:, 660:704], 0.0)
                        rs_b = _bc(rs[:, 0:1], [[1, HEADS], [0, F_HEAD]])
                        nc.vector.tensor_tensor(
                            A_norm[:, 0:660].rearrange("p (h f) -> p h f",
                                                       h=HEADS),
                            psum_g[:, 0:660].rearrange("p (h f) -> p h f",
                                                       h=HEADS),
                            rs_b, OP.mult)
                        psum_h1 = psh.tile([128, 330], F32, tag="psh1")
                        psum_h2 = psh.tile([128, 330], F32, tag="psh2")
                        for h in range(HEADS):
                            ph = psum_h1 if h < 5 else psum_h2
                            o = 66 * h - (0 if h < 5 else 330)
                            tp = pst.tile([96, 128], BF, tag="tp")
                            nc.tensor.transpose(
                                tp[:], A_norm[:, 66 * h:66 * h + 96],
                                ident_sb[:])
                            ahT = gw.tile([96, 128], BF, tag="ahT")
                            nc.vector.tensor_copy(ahT[:], tp[:])
                            nc.tensor.matmul(ph[:, o:o + 66], ahT[0:66, :],
                                             W_heads_sb[:, 66 * h:66 * (h + 1)],
                                             start=True, stop=False)
                            nc.tensor.matmul(ph[:, o:o + 66], ones_sb[0:1, 0:128],
                                             b_gat_sb[:, 66 * h:66 * (h + 1)],
                                             start=False, stop=True)
                        h_tile = gw.tile([128, 660], BF, tag="htile")
                        for half, ph in ((0, psum_h1), (1, psum_h2)):
                            th = gw.tile([128, 330], BF, tag="th")
                            nc.scalar.activation(th[:], ph[:, 0:330], AF.Copy)
                            u = gw.tile([128, 330], BF, tag="lku")
                            nc.vector.tensor_scalar(u[:], th[:], 0.01, None,
                                                    OP.mult)
                            nc.vector.tensor_tensor(
                                h_tile[:, 330 * half:330 * (half + 1)],
                                th[:], u[:], OP.max)
                        nc.sync.dma_start(
                            h_my[128 * t:128 * (t + 1), :], h_tile[:])

                # ---------------- AllGather ----------------
                if _rep == 0 or "AG" in rep_phases:
                    nc.gpsimd.collective_compute(
                        "AllGather", OP.bypass,
                        replica_groups=[core_ids],
                        ins=[h_my.opt()], outs=[h_full.opt()])

                # ---------------- phase C: GCN + feature-major MLP ------------
                if _rep == 0 or "C" in rep_phases:
                  with tc.tile_pool(name="phaseC", bufs=1) as pc, \
                     tc.tile_pool(name="hg_pool", bufs=8) as hgp, \
                     tc.tile_pool(name="gcn_work", bufs=4) as gcw, \
                     tc.tile_pool(name="grp", bufs=1) as grp, \
                     tc.tile_pool(name="psumC", bufs=2,
                                  space=bass.MemorySpace.PSUM) as psc, \
                     tc.tile_pool(name="psumT2", bufs=2,
                                  space=bass.MemorySpace.PSUM) as pst2, \
                     tc.tile_pool(name="psumM", bufs=2,
                                  space=bass.MemorySpace.PSUM) as psm:

                    wmasks_sb = pc.tile([128, nchunks * 128], BF)
                    halfw = (nchunks // 2) * 128
                    nc.sync.dma_start(wmasks_sb[:, 0:halfw],
                                      wmasks_d[:, 0:halfw])
                    nc.sync.dma_start(wmasks_sb[:, halfw:],
                                      wmasks_d[:, halfw:])
                    W_gcn_sb = pc.tile([128, 6 * GCN_OUT], BF)
                    nc.sync.dma_start(W_gcn_sb[:], W_gcn_d[:])
                    W_g1_sb = pc.tile([128, 11 * 1000], BF)
                    nc.sync.dma_start(W_g1_sb[:], W_g1_d[:])
                    W_g2_sb = pc.tile([128, 8 * 64], BF)
                    nc.sync.dma_start(W_g2_sb[:], W_g2_d[:])
                    b_gcn_sb = pc.tile([128, 11], F32)
                    nc.sync.dma_start(b_gcn_sb[:], b_gcn_col_d[:])
                    b_g1_sb = pc.tile([128, 8], F32)
                    nc.sync.dma_start(b_g1_sb[:], b_g1_col_d[:])
                    W_fc1_sb = pc.tile([64, 32], BF)
                    nc.sync.dma_start(W_fc1_sb[:], W_fc1_d[:])
                    W_fc2_sb = pc.tile([32, 16], BF)
                    nc.sync.dma_start(W_fc2_sb[:], W_fc2_d[:])
                    W_out_sb = pc.tile([16, 1], BF)
                    nc.sync.dma_start(W_out_sb[:], W_out_d[:])
                    b_tail_sb = pc.tile([128, 4], F32)
                    nc.sync.dma_start(b_tail_sb[:], b_tail_d[:])

                    def leaky_from_psum(ps_ap, out_ap, parts, nw, bias):
                        th = gcw.tile([128, 512], BF, tag="lk_t")
                        nc.scalar.activation(th[0:parts, 0:nw], ps_ap,
                                             AF.Identity, bias=bias)
                        u = gcw.tile([128, 512], BF, tag="lk_u")
                        nc.vector.tensor_scalar(u[0:parts, 0:nw],
                                                th[0:parts, 0:nw], 0.01, None,
                                                OP.mult)
                        nc.vector.tensor_tensor(out_ap, th[0:parts, 0:nw],
                                                u[0:parts, 0:nw], OP.max)

                    groups = [(0, 4), (4, 4), (8, 2)]
                    kws = [128] * 5 + [32]
                    kws1 = [128] * 10 + [40]
                    kws2 = [128] * 7 + [104]
                    for g0, gn in groups:
                        nw = gn * 128
                        aggT = grp.tile([128, 6 * 512], BF, tag="aggT")
                        for j in range(gn):
                            t = g0 + j
                            psum_a = psc.tile([128, F_GAT], F32, tag="psa")
                            for k in range(nc_t):
                                c = t * nc_t + k
                                hg = hgp.tile([128, 660], BF, tag="hg")
                                nc.gpsimd.indirect_dma_start(
                                    hg[:], None, h_full[:],
                                    IndirectOffsetOnAxis(
                                        ap=sidx_sb[:, c:c + 1], axis=0))
                                wmask = wmasks_sb[:, 128 * c:128 * (c + 1)]
                                st, sp = (k == 0), (k == nc_t - 1)
                                nc.tensor.matmul(psum_a[:, 0:512], wmask,
                                                 hg[:, 0:512], start=st, stop=sp)
                                nc.tensor.matmul(psum_a[:, 512:660], wmask,
                                                 hg[:, 512:660], start=st,
                                                 stop=sp)
                            agg = gcw.tile([128, 672], BF, tag="agg")
                            nc.scalar.activation(agg[:, 0:660], psum_a[:, 0:660],
                                                 AF.Copy)
                            nc.vector.memset(agg[:, 660:672], 0.0)
                            for b in range(6):
                                w = kws[b]
                                tp2 = pst2.tile([128, 128], BF, tag="tp2")
                                nc.tensor.transpose(tp2[0:w, :],
                                                    agg[:, 128 * b:128 * b + w],
                                                    ident_sb[:])
                                nc.vector.tensor_copy(
                                    aggT[0:w, 512 * b + 128 * j:
                                         512 * b + 128 * (j + 1)],
                                    tp2[0:w, :])

                        # ---- feature-major dense stack on this node group ----
                        gT = grp.tile([128, 11 * 512], BF, tag="gT")
                        for mc in range(11):
                            mw = 128 if mc < 10 else 40
                            ps = psm.tile([128, 512], F32, tag="psm")
                            for kt in range(6):
                                nc.tensor.matmul(
                                    ps[0:mw, 0:nw],
                                    W_gcn_sb[0:kws[kt],
                                             GCN_OUT * kt + 128 * mc:
                                             GCN_OUT * kt + 128 * mc + mw],
                                    aggT[0:kws[kt], 512 * kt:512 * kt + nw],
                                    start=(kt == 0), stop=(kt == 5))
                            leaky_from_psum(ps[0:mw, 0:nw],
                                            gT[0:mw, 512 * mc:512 * mc + nw],
                                            mw, nw, b_gcn_sb[0:mw, mc:mc + 1])

                        z1T = grp.tile([128, 8 * 512], BF, tag="z1T")
                        for mc in range(8):
                            mw = 128 if mc < 7 else 104
                            ps = psm.tile([128, 512], F32, tag="psm")
                            for kt in range(11):
                                nc.tensor.matmul(
                                    ps[0:mw, 0:nw],
                                    W_g1_sb[0:kws1[kt],
                                            1000 * kt + 128 * mc:
                                            1000 * kt + 128 * mc + mw],
                                    gT[0:kws1[kt], 512 * kt:512 * kt + nw],
                                    start=(kt == 0), stop=(kt == 10))
                            leaky_from_psum(ps[0:mw, 0:nw],
                                            z1T[0:mw, 512 * mc:512 * mc + nw],
                                            mw, nw, b_g1_sb[0:mw, mc:mc + 1])

                        ps2 = psm.tile([128, 512], F32, tag="psm")
                        for kt in range(8):
                            nc.tensor.matmul(
                                ps2[0:64, 0:nw],
                                W_g2_sb[0:kws2[kt], 64 * kt:64 * kt + 64],
                                z1T[0:kws2[kt], 512 * kt:512 * kt + nw],
                                start=(kt == 0), stop=(kt == 7))
                        z2T = gcw.tile([64, 512], BF, tag="z2T")
                        leaky_from_psum(ps2[0:64, 0:nw], z2T[0:64, 0:nw],
                                        64, nw, b_tail_sb[0:64, 0:1])

                        ps3 = psm.tile([128, 512], F32, tag="psm")
                        nc.tensor.matmul(ps3[0:32, 0:nw], W_fc1_sb[:],
                                         z2T[0:64, 0:nw], start=True, stop=True)
                        z3T = gcw.tile([32, 512], BF, tag="z3T")
                        leaky_from_psum(ps3[0:32, 0:nw], z3T[0:32, 0:nw],
                                        32, nw, b_tail_sb[0:32, 1:2])

                        ps4 = psm.tile([128, 512], F32, tag="psm")
                        nc.tensor.matmul(ps4[0:16, 0:nw], W_fc2_sb[:],
                                         z3T[0:32, 0:nw], start=True, stop=True)
                        z4T = gcw.tile([16, 512], BF, tag="z4T")
                        leaky_from_psum(ps4[0:16, 0:nw], z4T[0:16, 0:nw],
                                        16, nw, b_tail_sb[0:16, 2:3])

                        ps5 = psm.tile([128, 512], F32, tag="psm")
                        nc.tensor.matmul(ps5[0:1, 0:nw], W_out_sb[:],
                                         z4T[0:16, 0:nw], start=True, stop=True)
                        outT = gcw.tile([1, 512], F32, tag="outT")
                        nc.scalar.activation(outT[0:1, 0:nw], ps5[0:1, 0:nw],
                                             AF.Identity,
                                             bias=b_tail_sb[0:1, 3:4])
                        nc.sync.dma_start(y_d[0:1, 128 * g0:128 * g0 + nw],
                                          outT[0:1, 0:nw])

                # ---------------- timing probes (only when flagged) -------
                if "P" in rep_phases:
                    with tc.tile_pool(name="probeP", bufs=1) as ppr:
                        tp_ = ppr.tile([128, 128], BF)
                        nc.vector.memset(tp_[:], 0.0)
                if "Cg" in rep_phases or "Cg1" in rep_phases:
                    ng = 1 if "Cg1" in rep_phases else nchunks
                    with tc.tile_pool(name="probeCg", bufs=8) as pg_:
                        for c in range(ng):
                            hgp_ = pg_.tile([128, 660], BF, tag="hgprobe")
                            nc.gpsimd.indirect_dma_start(
                                hgp_[:], None, h_full[:],
                                IndirectOffsetOnAxis(
                                    ap=sidx_sb[:, c:c + 1], axis=0))
                if "Cd" in rep_phases:
                    with tc.tile_pool(name="probeCd", bufs=8) as pd_:
                        for c in range(nchunks):
                            cc = c % (NSLOT // 128)
                            hgd_ = pd_.tile([128, 660], BF, tag="hgdprobe")
                            nc.sync.dma_start(
                                hgd_[:], h_full[128 * cc:128 * (cc + 1), :])

    nc.compile()
    return nc


# ---------------------------------------------------------------- entry point

def kernel(x, edge_index, W_gat, att_src, att_dst, b_gat, W_gcn, b_gcn,
           W_g1, b_g1, W_g2, b_g2, W_fc1, b_fc1, W_fc2, b_fc2, W_out, b_out,
           _want_trace=False):
    x = np.asarray(x, np.float32)
    edge_index = np.asarray(edge_index)
    prep = _prep(x, edge_index)
    wts = _prep_weights(W_gat, att_src, att_dst, b_gat, W_gcn, b_gcn,
                        W_g1, b_g1, W_g2, b_g2, W_fc1, b_fc1, W_fc2, b_fc2,
                        W_out, b_out)

    nc_t = prep["nc_t"]
    if nc_t not in _CACHE:
        _CACHE[nc_t] = _build(nc_t)
    nc = _CACHE[nc_t]

    in_maps = make_in_maps(prep, wts)
    res = run_bass_kernel_spmd(nc, in_maps, list(range(NCORE)),
                               trace=_want_trace)
    y_all = np.concatenate([np.asarray(res.results[c]["y"]).reshape(-1)
                            for c in range(NCORE)])
    out = y_all[prep["slot"]].astype(np.float32).reshape(N, 1)
    if _want_trace:
        return out, res
    return out


if __name__ == "__main__":
    sys.path.insert(0, os.path.dirname(os.path.abspath(__file__)))
    import reference
    inputs = reference.setup_inputs()
    inputs = {k: np.asarray(v) for k, v in inputs.items()}
    expected = np.asarray(reference.reference(**inputs))
    got = kernel(**inputs)
    err = np.linalg.norm(got - expected) / np.linalg.norm(expected)
    print("Relative error:", err)
